# revision 62
# baseline (speedup 1.0000x reference)
"""Trainium2 Bass kernel for nn_MemoryAttention (causal single-head attention
with SiLU-gated output projection), sequence-parallel across 8 NeuronCores.

Strategy (per core c):
  - q rows owned: 4 slots of 256 rows: tile T = c + 8*s (strided assignment
    balances causal work; every core runs an identical instruction stream).
  - Everything hot runs in fp8e4 DoubleRow (2x PE throughput): Q/K
    projections and logits (host pre-scales wq/wk by 32; the exp() scale
    absorbs the extra 1024x), and the PV accumulation via a correction
    scheme: h = B + sum((p-1) * v8), where B = colsum of exact-V over the
    core's visible gathered blocks is a pure function of the inputs,
    computed on the host and uploaded as bf16 hi/lo pairs. Hidden blocks
    get weight 0 through a streamed 0/1 per-partition selector, so fp8
    V-quantization error only enters scaled by |p-1| ~ 0.15. The two
    diagonal (triangular-masked) blocks per slot stay on an exact bf16
    path. Row-sums get the matching constant via a per-core cinit upload.
  - Gathered kv blocks are processed in PAIRS so DoubleRow can contract
    256 kv rows per instruction; kv payloads (KT and V8) AllGather in four
    merged fp8 collectives (kt/v x group-halves) -- at these sizes the
    ~25us per-op latency floor dominates, so fewer bigger ops win.
  - Collective-wait windows are filled with locally recomputed copies of
    the first four kv blocks (dup) and the deferred half of the Q
    projection. V-pair loads ride the gpsimd SWDGE queue so PE-critical
    exp activations on the scalar queue never queue behind them.
  - All weight/x inputs are host-pre-permuted so every DMA is contiguous
    per partition (the naive layout costs ~1024 descriptors per load).
  - Slot epilogue: silu(h * (1/sums)) fused into one scalar activation,
    PE-transpose of G, output projection with G^T chunks stationary.
"""

import numpy as np
import ml_dtypes

import concourse.bass as bass
import concourse.tile as tile
from concourse import bacc, mybir
from concourse.bass_utils import run_bass_kernel_spmd
from concourse.masks import make_identity

P = 128
D = 1024
SEQ = 8192
NCORES = 8
NSLOTS = 4
QT_COLS = NSLOTS * 256
NBIN = 14  # gathered visits per slot that need a (binary) mask on some core
KSCALE = 32.0

F32 = mybir.dt.float32
BF16 = mybir.dt.bfloat16
F8 = mybir.dt.float8e4
AF = mybir.ActivationFunctionType
DR = mybir.MatmulPerfMode.DoubleRow
EXP_SCALE = 1.0 / (KSCALE * KSCALE * 32.0)  # logits carry 32q * 32k


def build_kernel():
    nc = bacc.Bacc(None, target_bir_lowering=False, num_devices=NCORES)

    # all weight/x params are host-pre-permuted so every DMA is contiguous
    # per partition (the naive [D, cols] layout costs ~1024 descriptors/DMA)
    xq8_ext = nc.declare_dram_parameter("xq8", [P, 4, 8, 256], F8, isOutput=False)
    xqb_ext = nc.declare_dram_parameter("xqb", [P, 4, 2, 8, P], BF16, isOutput=False)
    wq_ext = nc.declare_dram_parameter("wq8", [P, 8, 8, P], F8, isOutput=False)
    wk_ext = nc.declare_dram_parameter("wk8", [P, 8, 8, P], F8, isOutput=False)
    wv1_ext = nc.declare_dram_parameter("wv1", [P, 2, 8, 512], BF16, isOutput=False)
    wv2_ext = nc.declare_dram_parameter("wv2", [P, 8, 2, 512], BF16, isOutput=False)
    dmask_ext = nc.declare_dram_parameter("dmask", [P, 2, 256], BF16, isOutput=False)
    bias_ext = nc.declare_dram_parameter("bias", [P, NSLOTS * NBIN], F32, isOutput=False)
    bsum_ext = nc.declare_dram_parameter("bsum", [1, 2, NSLOTS, 2, 512], BF16, isOutput=False)
    cinit_ext = nc.declare_dram_parameter("cinit", [1, NSLOTS, 2], BF16, isOutput=False)
    xd8_ext = nc.declare_dram_parameter("xd8", [P, 2, 8, 256], F8, isOutput=False)
    xdb_ext = nc.declare_dram_parameter("xdb", [P, 2, 2, 8, P], BF16, isOutput=False)
    o_ext = nc.declare_dram_parameter("o", [NSLOTS, 2, P, D], F32, isOutput=True)

    # own kv payload per 256-col group g: [parity][p][m][c]. Separate tensors
    # per group: the tile framework tracks DRAM deps at tensor granularity,
    # so a shared tensor would delay the first gather until ALL groups project.
    v_loc = [nc.dram_tensor(f"v_loc{g}", [2, P, 8, P], BF16) for g in range(4)]
    kt_locA = nc.dram_tensor("kt_locA", [2, 2, P, 8, P], F8)
    v_locA = nc.dram_tensor("v_locA", [2, 2, P, 8, P], F8)
    kt_gA = nc.dram_tensor("kt_gA", [NCORES, 2, 2, P, 8, P], F8, addr_space="Shared")
    v_gA = nc.dram_tensor("v_gA", [NCORES, 2, 2, P, 8, P], F8, addr_space="Shared")
    # the B-half deadline is late, so kt+v merge into ONE op there
    kv_locB = nc.dram_tensor("kv_locB", [2, 2, 2, P, 8, P], F8)
    kv_gB = nc.dram_tensor("kv_gB", [NCORES, 2, 2, 2, P, 8, P], F8, addr_space="Shared")
    # locally re-projected copies of global blocks 0-3 (q-tiles 0/1): stall
    # filler that also defers the first gather consumption by four visits
    kt_dup = nc.dram_tensor("kt_dup", [4, P, 8, P], F8)
    v_dup = nc.dram_tensor("v_dup", [4, P, 8, P], F8)

    with tile.TileContext(nc) as tc:
        singles_ctx = tc.tile_pool(name="singles", bufs=1)
        singles = singles_ctx.__enter__()

        with (
            tc.tile_pool(name="projw", bufs=1) as projw,
            tc.tile_pool(name="projout", bufs=4) as projout,
            tc.tile_pool(name="ppsum", bufs=4, space="PSUM") as ppsum,
            tc.tile_pool(name="kpsum", bufs=4, space="PSUM") as kpsum,
        ):
            # loads ordered so the first kt-proj matmuls start early; spread
            # across both HWDGE queues (sync + scalar); all contiguous
            wv1_sb = projw.tile([P, 2, 8, 512], BF16, tag="wv1", name="wv1")
            nc.scalar.dma_start(out=wv1_sb[:, 0], in_=wv1_ext[:, 0])
            xqb_sb = projw.tile([P, 4, 2, 8, P], BF16, tag="xqb", name="xqb")
            nc.sync.dma_start(out=xqb_sb[:, 0], in_=xqb_ext[:, 0])
            wk_sb = projw.tile([P, 8, 8, P], F8, tag="wk", name="wk")
            xq8_sb = singles.tile([P, 4, 8, 256], F8)
            nc.sync.dma_start(out=xq8_sb[:, 0], in_=xq8_ext[:, 0])
            nc.sync.dma_start(out=wk_sb[:, :4], in_=wk_ext[:, :4])
            nc.scalar.dma_start(out=wv1_sb[:, 1], in_=wv1_ext[:, 1])
            nc.sync.dma_start(out=wk_sb[:, 4:], in_=wk_ext[:, 4:])
            for g in range(1, 4):
                nc.sync.dma_start(out=xq8_sb[:, g], in_=xq8_ext[:, g])
                nc.scalar.dma_start(out=xqb_sb[:, g], in_=xqb_ext[:, g])
            wq_sb = singles.tile([P, 8, 8, P], F8)
            nc.sync.dma_start(out=wq_sb, in_=wq_ext[:])
            xd8_sb = projw.tile([P, 2, 8, 256], F8, tag="xd8", name="xd8")
            nc.sync.dma_start(out=xd8_sb, in_=xd8_ext[:])
            xdb_sb = projw.tile([P, 2, 2, 8, P], BF16, tag="xdb", name="xdb")
            nc.scalar.dma_start(out=xdb_sb, in_=xdb_ext[:])

            ones_sb = singles.tile([P, 1], BF16)
            nc.vector.memset(ones_sb, 1.0)
            ones2_sb = singles.tile([P, 2, 1], F8)
            nc.vector.memset(ones2_sb, 1.0)
            onecol_sb = singles.tile([1, P], BF16)
            nc.vector.memset(onecol_sb, 1.0)
            zcol_sb = singles.tile([1, P], BF16)
            nc.vector.memset(zcol_sb, 0.0)
            zrow_sb = singles.tile([1, 2], BF16)
            nc.vector.memset(zrow_sb, 0.0)
            ident_sb = singles.tile([P, P], BF16)
            make_identity(nc, ident_sb)

            def gather(ins_ap, outs_ap):
                nc.gpsimd.collective_compute(
                    "AllGather",
                    mybir.AluOpType.bypass,
                    replica_groups=[list(range(NCORES))],
                    ins=[ins_ap],
                    outs=[outs_ap],
                )

            def kt_group(g):
                # KT for own group g: out [d_out(m,p), kv 256] in fp8
                dst = kt_locA[g] if g < 2 else kv_locB[0, g - 2]
                for m in range(8):
                    acc = kpsum.tile([P, 256], F32, tag="projk", name=f"kt{g}_{m}")
                    for sp in range(4):
                        nc.tensor.matmul(
                            acc,
                            lhsT=wk_sb[:, m, 2 * sp : 2 * sp + 2, :],
                            rhs=xq8_sb[:, g, 2 * sp : 2 * sp + 2, :],
                            start=(sp == 0),
                            stop=(sp == 3),
                            perf_mode=DR,
                        )
                    kt_out = projout.tile([P, 256], F8, tag="kt_out", name="kto")
                    nc.vector.tensor_copy(out=kt_out, in_=acc)
                    for par in range(2):
                        nc.sync.dma_start(
                            out=dst[par][:, m, :],
                            in_=kt_out[:, par * P : (par + 1) * P],
                        )

            def v_group(g):
                # V for own group g (bf16)
                for par in range(2):
                    v_out = projout.tile([P, 1024], BF16, tag="v_out", name="vo")
                    accs = [
                        ppsum.tile([P, 512], F32, tag="proj", name=f"vp{h2}")
                        for h2 in range(2)
                    ]
                    # h2-outer so the first chain only needs wv1's first half
                    for h2 in range(2):
                        for sub in range(8):
                            nc.tensor.matmul(
                                accs[h2],
                                lhsT=xqb_sb[:, g, par, sub, :],
                                rhs=wv1_sb[:, h2, sub, :],
                                start=(sub == 0),
                                stop=(sub == 7),
                            )
                    v_out8 = projout.tile([P, 1024], F8, tag="v_out8", name="vo8")
                    for h2 in range(2):
                        nc.vector.tensor_copy(
                            out=v_out[:, h2 * 512 : (h2 + 1) * 512], in_=accs[h2]
                        )
                        nc.vector.tensor_copy(
                            out=v_out8[:, h2 * 512 : (h2 + 1) * 512], in_=accs[h2]
                        )
                    nc.sync.dma_start(
                        out=v_loc[g][par].rearrange("p m c -> p (m c)"), in_=v_out
                    )
                    v8dst = v_locA[g] if g < 2 else kv_locB[1, g - 2]
                    nc.sync.dma_start(
                        out=v8dst[par].rearrange("p m c -> p (m c)"), in_=v_out8
                    )

            qt_sb = singles.tile([P, 8, QT_COLS], F8)

            def q_proj(g0, g1):
                # QT (fp8 DoubleRow) for 256-col groups g0..g1
                for m in range(8):
                    for g in range(g0, g1):
                        acc = kpsum.tile([P, 256], F32, tag="projk", name=f"q{g}_{m}")
                        for sp in range(4):
                            nc.tensor.matmul(
                                acc,
                                lhsT=wq_sb[:, m, 2 * sp : 2 * sp + 2, :],
                                rhs=xq8_sb[:, g, 2 * sp : 2 * sp + 2, :],
                                start=(sp == 0),
                                stop=(sp == 3),
                                perf_mode=DR,
                            )
                        nc.vector.tensor_copy(
                            out=qt_sb[:, m, g * 256 : (g + 1) * 256], in_=acc
                        )

            # 4 merged collectives: per-op latency floor (~25us) dominates at
            # these sizes, so fewer/bigger ops finish the stream sooner
            kt_group(0)
            kt_group(1)
            gather(kt_locA[:], kt_gA[:])
            v_group(0)
            v_group(1)
            gather(v_locA[:], v_gA[:])
            kt_group(2)
            v_group(2)
            kt_group(3)
            v_group(3)
            gather(kv_locB[:], kv_gB[:])

            # QT for slots 0/1; the rest is emitted inside the attention
            # section as collective-wait filler.
            q_proj(0, 2)

            # dup projection of global blocks 0-3 while the gathers fly
            for gg in range(2):
                for m in range(8):
                    acc = kpsum.tile([P, 256], F32, tag="projk", name=f"ktd{gg}_{m}")
                    for sp in range(4):
                        nc.tensor.matmul(
                            acc,
                            lhsT=wk_sb[:, m, 2 * sp : 2 * sp + 2, :],
                            rhs=xd8_sb[:, gg, 2 * sp : 2 * sp + 2, :],
                            start=(sp == 0),
                            stop=(sp == 3),
                            perf_mode=DR,
                        )
                    kt_out = projout.tile([P, 256], F8, tag="kt_out", name="ktod")
                    nc.vector.tensor_copy(out=kt_out, in_=acc)
                    for par in range(2):
                        nc.sync.dma_start(
                            out=kt_dup[2 * gg + par][:, m, :],
                            in_=kt_out[:, par * P : (par + 1) * P],
                        )
                for par in range(2):
                    v_out = projout.tile([P, 1024], F8, tag="v_out8", name="vod")
                    accs = [
                        ppsum.tile([P, 512], F32, tag="proj", name=f"vpd{h2}")
                        for h2 in range(2)
                    ]
                    for h2 in range(2):
                        for sub in range(8):
                            nc.tensor.matmul(
                                accs[h2],
                                lhsT=xdb_sb[:, gg, par, sub, :],
                                rhs=wv1_sb[:, h2, sub, :],
                                start=(sub == 0),
                                stop=(sub == 7),
                            )
                    for h2 in range(2):
                        nc.vector.tensor_copy(
                            out=v_out[:, h2 * 512 : (h2 + 1) * 512], in_=accs[h2]
                        )
                    nc.sync.dma_start(
                        out=v_dup[2 * gg + par].rearrange("p m c -> p (m c)"),
                        in_=v_out,
                    )

        # ---- attention ----------------------------------------------------
        with (
            tc.tile_pool(name="asingles", bufs=1) as asingles,
            tc.tile_pool(name="vpool", bufs=10) as vpool,
            tc.tile_pool(name="epool", bufs=2) as epool,
            tc.tile_pool(name="gpool", bufs=2) as gpool,
            tc.tile_pool(name="ltpsum", bufs=2, space="PSUM") as ltpsum,
            tc.tile_pool(name="hpsum", bufs=1, space="PSUM") as hpsum,
            tc.tile_pool(name="spsum", bufs=1, space="PSUM") as spsum,
            tc.tile_pool(name="tppsum", bufs=1, space="PSUM") as tppsum,
        ):
            wv2_sb = asingles.tile([P, 8, 2, 512], BF16, tag="wv2", name="wv2")
            nc.sync.dma_start(out=wv2_sb, in_=wv2_ext[:])
            dm_sb = asingles.tile([P, 2, 256], BF16, tag="dm", name="dm")
            nc.sync.dma_start(out=dm_sb, in_=dmask_ext[:])
            bias_sb = asingles.tile([P, NSLOTS * NBIN], F32, tag="bias", name="bias")
            nc.sync.dma_start(out=bias_sb, in_=bias_ext[:])
            actscratch = asingles.tile([P, 1], BF16, tag="actp", name="actp")
            bsum_sb = asingles.tile([1, 2, NSLOTS, 2, 512], BF16, tag="bsum", name="bsum")
            nc.sync.dma_start(out=bsum_sb, in_=bsum_ext[:])
            cinit_sb = asingles.tile([1, NSLOTS, 2], BF16, tag="cinit", name="cinit")
            nc.sync.dma_start(out=cinit_sb, in_=cinit_ext[:])

            def visit_srcs(s, kind, idx):
                if kind == "diag":
                    kt_src = kt_locA[s, idx] if s < 2 else kv_locB[0, s - 2, idx]
                    return kt_src, v_loc[s][idx]
                if idx < 4:
                    return kt_dup[idx], v_dup[idx]
                g, src, par = idx // 16, (idx % 16) // 2, idx % 2
                if g < 2:
                    return kt_gA[src, g, par], v_gA[src, g, par]
                return kv_gB[src, 0, g - 2, par], kv_gB[src, 1, g - 2, par]

            def q_proj_filler():
                # QT cols 512:1024 (slots 2/3), emitted as stall filler while
                # slot 0 waits for the first gathers; accumulates in the lt pool
                for m in range(8):
                    for g in range(2, 4):
                        acc = ltpsum.tile([P, 256], F32, tag="lt", name=f"qf{m}_{g}")
                        for sp in range(4):
                            nc.tensor.matmul(
                                acc,
                                lhsT=wq_sb[:, m, 2 * sp : 2 * sp + 2, :],
                                rhs=xq8_sb[:, g, 2 * sp : 2 * sp + 2, :],
                                start=(sp == 0),
                                stop=(sp == 3),
                                perf_mode=DR,
                            )
                        nc.vector.tensor_copy(
                            out=qt_sb[:, m, g * 256 : (g + 1) * 256], in_=acc
                        )

            def load_diag(s, d):
                kt_src = kt_locA[s, d] if s < 2 else kv_locB[0, s - 2, d]
                kt_t = vpool.tile([P, 8, P], F8, tag="kt", name="kt_t")
                nc.sync.dma_start(out=kt_t, in_=kt_src)
                v_t = vpool.tile([P, 1024], BF16, tag="v", name="v_t")
                nc.gpsimd.dma_start(
                    out=v_t, in_=v_loc[s][d].rearrange("p m c -> p (m c)")
                )
                return kt_t, v_t

            def pair_srcs(idx):
                # blocks (idx, idx+1) are parity 0/1 of one source: [2, P, 8, P]
                if idx < 4:
                    return kt_dup[idx : idx + 2], v_dup[idx : idx + 2]
                g, src = idx // 16, (idx % 16) // 2
                if g < 2:
                    return kt_gA[src, g], v_gA[src, g]
                return kv_gB[src, 0, g - 2], kv_gB[src, 1, g - 2]

            def load_pair(s, idx):
                # one DMA per operand pair: halves the per-pair issue cost on
                # the sync queue (2x650ns was ~80% of the pair cadence)
                kt_src2, v_src2 = pair_srcs(idx)
                kt2 = vpool.tile([P, 2, 8, P], F8, tag="kt", name="kt2")
                nc.sync.dma_start(out=kt2, in_=kt_src2.rearrange("q p m c -> p q m c"))
                # gpsimd (SWDGE) queue: keeps the collective-gated v loads off
                # the scalar queue, whose exps are PE-critical (HOL blocking)
                v2 = vpool.tile([P, 2, 1024], F8, tag="v8", name="v2")
                nc.gpsimd.dma_start(
                    out=v2, in_=v_src2.rearrange("q p m c -> p q (m c)")
                )
                return kt2, v2

            def logits(s, kt_t):
                lt = ltpsum.tile([P, 256], F32, tag="lt", name="lt")
                for sp in range(4):
                    nc.tensor.matmul(
                        lt,
                        lhsT=kt_t[:, 2 * sp : 2 * sp + 2, :],
                        rhs=qt_sb[:, 2 * sp : 2 * sp + 2, s * 256 : (s + 1) * 256],
                        start=(sp == 0),
                        stop=(sp == 3),
                        perf_mode=DR,
                    )
                return lt

            def pv_diag(s, d, lt, v_t, h, sums, first):
                # exact bf16 path for the diagonal pair (p weights, not p-1)
                pt = vpool.tile([P, 256], BF16, tag="pt", name="ptd")
                nc.scalar.activation(out=pt, in_=lt, func=AF.Exp, scale=EXP_SCALE)
                nc.vector.tensor_mul(out=pt, in0=pt, in1=dm_sb[:, d, :])
                for qc in range(2):
                    lhsT = pt[:, qc * P : (qc + 1) * P]
                    for dh in range(2):
                        nc.tensor.matmul(
                            h[qc][:, dh, :],
                            lhsT=lhsT,
                            rhs=v_t[:, dh * 512 : (dh + 1) * 512],
                            start=first,
                            stop=False,
                        )
                    nc.tensor.matmul(
                        sums[:, qc : qc + 1],
                        lhsT=lhsT,
                        rhs=ones_sb,
                        start=False,
                        stop=False,
                        skip_group_check=True,
                    )

            def pv_pair(s, idx, lts, v2, h, sums, is_last):
                # fp8 DoubleRow correction term: weights are p-1 (0 for
                # hidden blocks via the 0/1 selector); the bulk sum of V over
                # visible blocks arrives via the host-computed bsum constant
                ptf = vpool.tile([P, 2, 256], BF16, tag="ptf", name="ptf")
                for half in range(2):
                    nc.scalar.activation(
                        out=ptf[:, half, :], in_=lts[half], func=AF.Exp,
                        scale=EXP_SCALE,
                    )
                pt8 = vpool.tile([P, 2, 256], F8, tag="pt8", name="pt8")
                if idx + 1 >= 16 * s:
                    for half in range(2):
                        b = idx + half
                        if b >= 16 * s:
                            mi = s * NBIN + (b - 16 * s)
                            nc.vector.tensor_scalar(
                                out=pt8[:, half, :],
                                in0=ptf[:, half, :],
                                scalar1=-1.0,
                                scalar2=bias_sb[:, mi : mi + 1],
                                op0=mybir.AluOpType.add,
                                op1=mybir.AluOpType.mult,
                            )
                        else:
                            nc.vector.tensor_scalar_add(
                                out=pt8[:, half, :], in0=ptf[:, half, :],
                                scalar1=-1.0,
                            )
                else:
                    nc.vector.tensor_scalar_add(out=pt8, in0=ptf, scalar1=-1.0)
                for qc in range(2):
                    lhsT = pt8[:, :, qc * P : (qc + 1) * P]
                    for dh in range(2):
                        nc.tensor.matmul(
                            h[qc][:, dh, :],
                            lhsT=lhsT,
                            rhs=v2[:, :, dh * 512 : (dh + 1) * 512],
                            start=False,
                            stop=is_last,
                            perf_mode=DR,
                        )
                    nc.tensor.matmul(
                        sums[:, qc : qc + 1],
                        lhsT=lhsT,
                        rhs=ones2_sb,
                        start=False,
                        stop=is_last,
                        skip_group_check=True,
                        perf_mode=DR,
                    )

            for s in range(NSLOTS):
                gp = [2 * b for b in range(8 * s + 7)]
                h = [
                    hpsum.tile([P, 2, 512], F32, tag=f"hq{qc}", name=f"h{qc}_{s}")
                    for qc in range(2)
                ]
                sums = spsum.tile([P, 2], F32, tag="sums", name="sums")
                # init sums with C = 128 * n_visible_gathered (per-core data)
                nc.tensor.matmul(
                    sums,
                    lhsT=onecol_sb,
                    rhs=cinit_sb[:, s, :],
                    start=True,
                    stop=False,
                    skip_group_check=True,
                )
                # diagonal pair on the exact bf16 path
                ktd0, vd0 = load_diag(s, 0)
                lt0 = logits(s, ktd0)
                ktd1, vd1 = load_diag(s, 1)
                lt1 = logits(s, ktd1)
                pv_diag(s, 0, lt0, vd0, h, sums, True)
                # gathered pairs: software pipeline, logits of j+1 before pv of j
                kt2, v2 = load_pair(s, gp[0])
                lts_prev = (logits(s, kt2[:, 0]), logits(s, kt2[:, 1]))
                v_prev = v2
                idx_prev = gp[0]
                pv_diag(s, 1, lt1, vd1, h, sums, False)
                # add the host-computed bulk term B (hi + lo bf16 halves) now:
                # accumulation commutes, and early placement keeps it off the
                # last-pv -> silu critical chain at the end of the slot
                for t in range(2):
                    for qc in range(2):
                        for dh in range(2):
                            nc.tensor.matmul(
                                h[qc][:, dh, :],
                                lhsT=onecol_sb,
                                rhs=bsum_sb[:, t, s, dh, :],
                                start=False,
                                stop=False,
                                skip_group_check=True,
                            )
                if s == 0:
                    q_proj_filler()
                for j in range(1, len(gp)):
                    kt2, v2 = load_pair(s, gp[j])
                    lts = (logits(s, kt2[:, 0]), logits(s, kt2[:, 1]))
                    pv_pair(s, idx_prev, lts_prev, v_prev, h, sums, False)
                    lts_prev, v_prev, idx_prev = lts, v2, gp[j]
                pv_pair(s, idx_prev, lts_prev, v_prev, h, sums, True)
                # prefetch the Silu activation table while the PE drains the
                # last matmuls (the table switch costs ~4.5us on-critical)
                nc.scalar.activation(out=actscratch, in_=ones_sb, func=AF.Silu)

                # ---- epilogue (chained per qc so qc0's output projection
                # overlaps qc1's vector/scalar work) -----------------------
                gt_sb = epool.tile([P, 8, 256], BF16, tag="gt", name="gt")
                for qc in range(2):
                    recip = epool.tile([P, 1], F32, tag="recip", name="recip")
                    nc.vector.reciprocal(out=recip, in_=sums[:, qc : qc + 1])
                    gb = gpool.tile([P, 1024], BF16, tag=f"g{qc}", name=f"g{qc}")
                    # silu(h / sums) in one op: per-partition recip as act scale
                    nc.scalar.activation(
                        out=gb,
                        in_=h[qc].rearrange("p a b -> p (a b)"),
                        func=AF.Silu,
                        scale=recip,
                    )
                    # transpose G -> gt [d-part, m, qc*P:...]
                    for m in range(8):
                        tp = tppsum.tile([P, 256], BF16, tag="tp", name="tp")
                        nc.tensor.transpose(
                            tp[:, :P],
                            gb[:, m * P : (m + 1) * P],
                            ident_sb,
                        )
                        nc.vector.tensor_copy(
                            out=gt_sb[:, m, qc * P : (qc + 1) * P], in_=tp[:, :P]
                        )
                    # output projection: O[q, d] via lhsT = gt chunks
                    op = hpsum.tile(
                        [P, 2, 512], F32, tag=f"hq{qc}", name=f"o{qc}_{s}"
                    )
                    for m in range(8):
                        for dh in range(2):
                            nc.tensor.matmul(
                                op[:, dh, :],
                                lhsT=gt_sb[:, m, qc * P : (qc + 1) * P],
                                rhs=wv2_sb[:, m, dh, :],
                                start=(m == 0),
                                stop=(m == 7),
                            )
                    oo = epool.tile([P, 2, 512], F32, tag="oo", name="oo")
                    nc.vector.tensor_copy(out=oo, in_=op)
                    nc.gpsimd.dma_start(
                        out=o_ext[s, qc], in_=oo.rearrange("p a b -> p (a b)")
                    )
                if s < NSLOTS - 1:
                    nc.scalar.activation(out=actscratch, in_=ones_sb, func=AF.Exp)

        singles_ctx.__exit__(None, None, None)

    nc.finalize()
    return nc


_NC_CACHE = {}


def get_nc():
    if "nc" not in _NC_CACHE:
        _NC_CACHE["nc"] = build_kernel()
    return _NC_CACHE["nc"]


def build_dmask():
    p = np.arange(P)[:, None]
    u = np.arange(256)[None, :]
    m0 = (p <= u).astype(np.float32)
    m1 = (p + P <= u).astype(np.float32)
    return np.stack([m0, m1], axis=1).astype(ml_dtypes.bfloat16)  # [P, 2, 256]


def build_msel(c):
    """multiplicative visibility selector per gathered-tail visit: slot s,
    i = block - 16s in [0, 14): visible on core c iff i < 2c."""
    row = np.empty(NSLOTS * NBIN, np.float32)
    for s in range(NSLOTS):
        for i in range(NBIN):
            row[s * NBIN + i] = 1.0 if i < 2 * c else 0.0
    return np.broadcast_to(row, (P, NSLOTS * NBIN)).copy()


def build_bsum_cinit(x32, wv1, c):
    """bulk V sums over gathered-visible blocks, per (core, slot), bf16 hi/lo."""
    bf = ml_dtypes.bfloat16
    xsum = x32.reshape(64, P, D).sum(1)  # [64, 1024]
    wv1b = wv1.astype(bf).astype(np.float32)
    bsum = np.zeros((1, 2, NSLOTS, 2, 512), np.float32)
    cinit = np.zeros((1, NSLOTS, 2), np.float32)
    for s in range(NSLOTS):
        nvis = 16 * s + min(2 * c, NBIN)
        cinit[0, s, :] = 128.0 * nvis
        if nvis:
            B = xsum[:nvis].sum(0) @ wv1b  # [1024]
            hi = B.astype(bf).astype(np.float32)
            lo = (B - hi).astype(bf).astype(np.float32)
            bsum[0, 0, s] = hi.reshape(2, 512)
            bsum[0, 1, s] = lo.reshape(2, 512)
    return bsum.astype(bf), cinit.astype(bf)


def build_in_maps(x, wq, wk, wv1, wv2):
    bf = ml_dtypes.bfloat16
    f8 = ml_dtypes.float8_e4m3fn
    x32 = np.asarray(x, np.float32)
    wv1_32 = np.asarray(wv1, np.float32)
    xT = np.ascontiguousarray(x32.T)
    dmask = build_dmask()

    def to8(a):
        return np.clip(np.asarray(a, np.float32), -240, 240).astype(f8)

    def perm_w(w):  # [D, D] -> [p, m, sub, c]
        return np.ascontiguousarray(
            np.asarray(w, np.float32).reshape(8, P, 8, P).transpose(1, 2, 0, 3)
        )

    xd = xT[:, :512]
    w = {
        "wq8": to8(perm_w(np.asarray(wq, np.float32) * KSCALE)),
        "wk8": to8(perm_w(np.asarray(wk, np.float32) * KSCALE)),
        # wv1: [p, h2, sub, s]; wv2: [p, m, dh, s]
        "wv1": np.ascontiguousarray(
            np.asarray(wv1, np.float32).reshape(8, P, 2, 512).transpose(1, 2, 0, 3)
        ).astype(bf),
        "wv2": np.ascontiguousarray(
            np.asarray(wv2, np.float32).reshape(8, P, 2, 512).transpose(1, 0, 2, 3)
        ).astype(bf),
        "dmask": dmask,
        # xd8: [p, g, sub, s]; xdb: [p, g, par, sub, c]
        "xd8": to8(xd.reshape(8, P, 2, 256).transpose(1, 2, 0, 3)),
        "xdb": np.ascontiguousarray(
            xd.reshape(8, P, 2, 2, P).transpose(1, 2, 3, 0, 4)
        ).astype(bf),
    }
    in_maps = []
    for c in range(NCORES):
        xq_c = np.concatenate(
            [xT[:, 256 * (c + 8 * s) : 256 * (c + 8 * s) + 256] for s in range(NSLOTS)],
            axis=1,
        )
        bsum, cinit = build_bsum_cinit(x32, wv1_32, c)
        in_maps.append(
            {
                "xq8": to8(xq_c.reshape(8, P, 4, 256).transpose(1, 2, 0, 3)),
                "xqb": np.ascontiguousarray(
                    xq_c.reshape(8, P, 4, 2, P).transpose(1, 2, 3, 0, 4)
                ).astype(bf),
                "bias": build_msel(c),
                "bsum": bsum,
                "cinit": cinit,
                **w,
            }
        )
    return in_maps


def assemble_out(results):
    out = np.empty((SEQ, D), np.float32)
    for c in range(NCORES):
        o = results[c]["o"]  # [4, 2, 128, 1024]
        for s in range(NSLOTS):
            r0 = 256 * (c + 8 * s)
            out[r0 : r0 + P, :] = o[s, 0]
            out[r0 + P : r0 + 256, :] = o[s, 1]
    return out


def kernel(x, wq, wk, wv1, wv2):
    in_maps = build_in_maps(x, wq, wk, wv1, wv2)
    nc = get_nc()
    res = run_bass_kernel_spmd(nc, in_maps, list(range(NCORES)))
    return assemble_out(res.results)
